# revision 31
# baseline (speedup 1.0000x reference)
"""Bahdanau attention kernel for 8 TRN2 NeuronCores.

Data-parallel over batch: each core handles 16 of the 128 batches.
Per core (m = 16*196 = 3136 flattened (batch, n) rows), pipelined over
4 groups of 4 batches:
  phase 1(g): att1^T[a, m] = W_enc^T @ enc^T  (bf16 matmuls),
              fused bias+relu on ScalarE (att2 per-batch segments),
              e[m] = W_full^T @ combined  (PE matvec, deferred one
              P-block so it never stalls the in-order PE stream)
  softmax(g): per batch over n on partition 0 (overlaps the next group)
  phase 2:    awe[b, :] = A^T @ enc_flat with A = block-diag(alpha),
              single accumulation pass at the end; encN prefetch triggers
              spread through phase 1.
DMA triggers cost ~0.7us each on the sync engine, so loads are merged:
4 e-chunks per encT trigger, one trigger for wdec/decT.
All heavy matmuls run in bf16 (host-cast); accumulation is f32 PSUM.
"""

import numpy as np

CORES = 8
B, N = 128, 196
ENC, DEC, ATT = 2048, 512, 512
BPC = B // CORES          # 16 batches per core
M = BPC * N               # 3136 flattened rows per core
GROUPS = 4
BPG = BPC // GROUPS       # 4 batches per group
GM = BPG * N              # 784 rows per group
MCH = 512                 # phase-1 moving chunk (PSUM bank limit, f32 out)
ECH = ENC // 128          # 16 contraction chunks
ACH = ATT // 128          # 4 a-chunks
DCH = DEC // 128          # 4 d-chunks
EQ = 4                    # e-chunks per merged encT DMA


def _chunks(base, total, step):
    return [(base + o, min(step, total - o)) for o in range(0, total, step)]


def _segments(off, w):
    """(batch, start, end) segments (chunk-local cols) of flat range."""
    segs = []
    b0, b1 = off // N, (off + w - 1) // N
    for b in range(b0, b1 + 1):
        s0, s1 = max(off, b * N), min(off + w, (b + 1) * N)
        segs.append((b, s0 - off, s1 - off))
    return segs


def build_graph():
    import concourse.bacc as bacc
    import concourse.mybir as mybir
    import concourse.tile as tile

    f32 = mybir.dt.float32
    bf16 = mybir.dt.bfloat16
    AF = mybir.ActivationFunctionType

    nc = bacc.Bacc("TRN2", target_bir_lowering=False, debug=False)

    encT = nc.dram_tensor("encT", [ENC, M], bf16, kind="ExternalInput").ap()
    encN = nc.dram_tensor("encN", [M, ENC], bf16, kind="ExternalInput").ap()
    decT = nc.dram_tensor("decT", [DEC, BPC], bf16, kind="ExternalInput").ap()
    wenc = nc.dram_tensor("wenc", [ENC, ATT], bf16, kind="ExternalInput").ap()
    wdec = nc.dram_tensor("wdec", [DEC, ATT], bf16, kind="ExternalInput").ap()
    wfull = nc.dram_tensor("wfull", [128, ACH], bf16, kind="ExternalInput").ap()
    bsum = nc.dram_tensor("bsum", [1, ATT], bf16, kind="ExternalInput").ap()
    ident = nc.dram_tensor("ident", [BPC, BPC], bf16, kind="ExternalInput").ap()
    alpha_out = nc.dram_tensor("alpha_out", [BPC, N], f32, kind="ExternalOutput").ap()
    awe_out = nc.dram_tensor("awe_out", [BPC, ENC], f32, kind="ExternalOutput").ap()

    p2ch = _chunks(0, M, 128)          # 25 phase-2 chunks
    nch2 = len(p2ch)
    W0 = GM - MCH                      # 272: group-0's narrow chunk first,
    p1ch = [(0, 0, W0), (0, W0, MCH)]  # shrinking the HBM-bound startup set
    for g in range(1, GROUPS):
        for off, w in _chunks(GM * g, GM, MCH):
            p1ch.append((g, off, w))

    with tile.TileContext(nc) as tc:
        with (
            tc.tile_pool(name="consts", bufs=1) as consts,
            tc.tile_pool(name="xT", bufs=12) as xt_pool,
            tc.tile_pool(name="xN", bufs=nch2) as xn_pool,
            tc.tile_pool(name="Apool", bufs=nch2) as a_pool,
            tc.tile_pool(name="comb", bufs=6) as comb_pool,
            tc.tile_pool(name="sm", bufs=2) as sm_pool,
        ):
            def load_xt(off, w, tag_sfx):
                """4 merged DMAs, each covering 4 e-chunks of encT cols.
                Returns a slicer f(e, w) -> rhs AP."""
                quarters = []
                for q in range(EQ):
                    t = xt_pool.tile([128, EQ * MCH], bf16, tag="xt",
                                     name=f"xt_{tag_sfx}_{q}")
                    src = encT[512 * q:512 * q + 512, off:off + w].rearrange(
                        "(e p) c -> p e c", p=128)
                    dst = t[:].rearrange("p (e c) -> p e c", e=EQ)[:, :, 0:w]
                    nc.sync.dma_start(dst, src)
                    quarters.append(t)
                return lambda e, w: quarters[e // EQ][
                    :, MCH * (e % EQ):MCH * (e % EQ) + w]

            # PE warmup: the HAM clock gate needs ~3.4us of sustained
            # activity to lift the 1.2GHz throttle. Dummy matmuls on a
            # memset tile run while the first real loads are in flight, so
            # the real stream starts at 2.4GHz. The memset is the first DVE
            # instruction so the warmup isn't delayed by other memsets.
            wu_sb = consts.tile([128, MCH], bf16, tag="wu")
            nc.vector.memset(wu_sb[:], 0.25)
            with tc.tile_pool(name="wupsum", bufs=1, space="PSUM") as wu_pool:
                wu_ps = wu_pool.tile([128, MCH], f32, tag="wups")
                for _ in range(5):
                    nc.tensor.matmul(wu_ps[:], lhsT=wu_sb[:, 0:128],
                                     rhs=wu_sb[:], start=True, stop=True)

            # ---- startup: first e-chunks land as small single DMAs so the
            # first matmul starts as early as possible ----
            wenc_fine = []
            xt0_fine = []
            for e in range(EQ):
                t = consts.tile([128, ATT], bf16, tag=f"wencf{e}",
                                name=f"wenc_fine{e}")
                nc.sync.dma_start(t[:], wenc[128 * e:128 * e + 128, :])
                wenc_fine.append(t)
                t2 = xt_pool.tile([128, MCH], bf16, tag="xtf", bufs=4,
                                  name=f"xt0_fine{e}")
                nc.sync.dma_start(t2[:, 0:W0],
                                  encT[128 * e:128 * e + 128, 0:W0])
                xt0_fine.append(t2)

            wenc_sb = [None]
            xt0_q = [None]
            for q in range(1, EQ):
                t = consts.tile([128, EQ * ATT], bf16, tag=f"wenc{q}",
                                name=f"wenc_sb{q}")
                src = wenc[512 * q:512 * q + 512, :].rearrange(
                    "(e p) c -> p e c", p=128)
                dst = t[:].rearrange("p (e c) -> p e c", e=EQ)
                nc.sync.dma_start(dst, src)
                wenc_sb.append(t)
                tq = xt_pool.tile([128, EQ * MCH], bf16, tag="xt",
                                  name=f"xt0_q{q}")
                srcq = encT[512 * q:512 * q + 512, 0:W0].rearrange(
                    "(e p) c -> p e c", p=128)
                dstq = tq[:].rearrange("p (e c) -> p e c", e=EQ)[:, :, 0:W0]
                nc.sync.dma_start(dstq, srcq)
                xt0_q.append(tq)

            def wenc_slice(e, a):
                if e < EQ:
                    return wenc_fine[e][:, 128 * a:128 * a + 128]
                return wenc_sb[e // EQ][:, ATT * (e % EQ) + 128 * a:
                                        ATT * (e % EQ) + 128 * a + 128]

            def xt0_slice(e, w):
                if e < EQ:
                    return xt0_fine[e][:, 0:w]
                return xt0_q[e // EQ][:, MCH * (e % EQ):MCH * (e % EQ) + w]

            decT_sb = consts.tile([128, DCH * BPC], bf16, tag="decT")
            nc.sync.dma_start(
                decT_sb[:].rearrange("p (d c) -> p d c", d=DCH),
                decT[:].rearrange("(d p) c -> p d c", p=128))
            wdec_sb = consts.tile([128, DCH * ATT], bf16, tag="wdec")
            nc.sync.dma_start(
                wdec_sb[:].rearrange("p (d c) -> p d c", d=DCH),
                wdec[:].rearrange("(d p) c -> p d c", p=128))

            late = {}
            late["bsum"] = consts.tile([1, ATT], bf16, tag="bsum",
                                       name="bsum_sb")
            nc.sync.dma_start(late["bsum"][:], bsum[:])
            late["wfull"] = consts.tile([128, ACH], bf16, tag="wfull",
                                        name="wfull_sb")
            nc.sync.dma_start(late["wfull"][:], wfull[:])

            ones_sb = consts.tile([1, BPC], bf16, tag="ones")
            nc.vector.memset(ones_sb[:], 1.0)
            AT = consts.tile([BPC, M], bf16, tag="AT")
            nc.vector.memset(AT[:], 0.0)

            # ident is needed only by the group-1 transposes — issue its
            # trigger after chunk 1's encT loads
            def emit_late_consts():
                late["ident"] = consts.tile([BPC, BPC], bf16, tag="ident",
                                            name="ident_sb")
                nc.sync.dma_start(late["ident"][:], ident[:])

            A_tiles = [None] * nch2
            xn_tiles = [None] * nch2
            att2T_sb = []
            pending_e = []          # deferred e-matmuls
            xn_next = [0]           # next phase-2 chunk to prefetch
            a_done = [0]            # transposes emitted so far

            def emit_pending_e():
                while pending_e:
                    pending_e.pop(0)()

            def emit_att2():
                with tc.tile_pool(name="t2psum", bufs=1,
                                  space="PSUM") as t2_pool:
                    for aa in range(ACH):
                        t2ps = t2_pool.tile([128, BPC], f32, tag="t2",
                                            name=f"t2ps{aa}")
                        for d in range(DCH):
                            nc.tensor.matmul(
                                t2ps[:],
                                lhsT=wdec_sb[:, ATT * d + 128 * aa:
                                             ATT * d + 128 * aa + 128],
                                rhs=decT_sb[:, BPC * d:BPC * d + BPC],
                                start=(d == 0), stop=False,
                            )
                        nc.tensor.matmul(
                            t2ps[:],
                            lhsT=late["bsum"][0:1, 128 * aa:128 * aa + 128],
                            rhs=ones_sb[0:1, :],
                            start=False, stop=True,
                        )
                        t = consts.tile([128, BPC], f32, tag=f"att2T{aa}",
                                        name=f"att2T_sb{aa}")
                        nc.vector.tensor_copy(t[:], t2ps[:])
                        att2T_sb.append(t)

            def prefetch_xn(count):
                while count > 0 and xn_next[0] < nch2:
                    jj = xn_next[0]
                    off2, w2 = p2ch[jj]
                    t = xn_pool.tile([128, ENC], bf16, tag="xn",
                                     name=f"xn{jj}")
                    nc.sync.dma_start(t[:w2, :], encN[off2:off2 + w2, :])
                    xn_tiles[jj] = t
                    xn_next[0] += 1
                    count -= 1

            def emit_transposes(at_pool, lim):
                while (a_done[0] < nch2
                       and p2ch[a_done[0]][0] + p2ch[a_done[0]][1] <= lim):
                    off2, w2 = p2ch[a_done[0]]
                    aps = at_pool.tile([128, BPC], bf16, tag="Aps",
                                       name=f"aps{a_done[0]}")
                    nc.tensor.transpose(
                        aps[:w2, :], AT[:, off2:off2 + w2], late["ident"][:])
                    t = a_pool.tile([128, BPC], bf16, tag="A",
                                    name=f"A_sb{a_done[0]}")
                    nc.vector.tensor_copy(t[:w2, :], aps[:w2, :])
                    A_tiles[a_done[0]] = t
                    a_done[0] += 1

            at_ctx = tc.tile_pool(name="atpsum", bufs=1, space="PSUM")
            at_pool = at_ctx.__enter__()
            with (
                tc.tile_pool(name="p1psum", bufs=4, space="PSUM") as p1_pool,
                tc.tile_pool(name="epsum", bufs=2, space="PSUM") as e_pool,
            ):
                e_rows = {}
                for ki, (g, off, w) in enumerate(p1ch):
                    gbase = GM * g
                    if ki == 0:
                        xt = xt0_slice
                    else:
                        xt = load_xt(off, w, f"k{ki}")
                        if ki == 1:
                            emit_late_consts()
                    if g not in e_rows:
                        e_rows[g] = sm_pool.tile([1, GM], f32, tag="e_row",
                                                 name=f"e_row{g}")
                    e_row = e_rows[g]
                    e_ps = e_pool.tile([1, MCH], f32, tag="eps",
                                       name=f"eps{ki}")
                    if ki == 0:
                        # e-outer: each e-step needs only 256KB of fresh
                        # data, so the PE never outruns the startup DMAs
                        Ps = [p1_pool.tile([128, MCH], f32, tag="P",
                                           name=f"P0_{a}")
                              for a in range(ACH)]
                        for e in range(ECH):
                            for a in range(ACH):
                                nc.tensor.matmul(
                                    Ps[a][:, :w],
                                    lhsT=wenc_slice(e, a),
                                    rhs=xt(e, w),
                                    start=(e == 0), stop=(e == ECH - 1),
                                )
                            if e == 7:
                                emit_att2()
                    for a in range(ACH):
                        if ki == 0:
                            P = Ps[a]
                        else:
                            P = p1_pool.tile([128, MCH], f32, tag="P",
                                             name=f"P{ki}_{a}")
                            for e in range(ECH):
                                nc.tensor.matmul(
                                    P[:, :w],
                                    lhsT=wenc_slice(e, a),
                                    rhs=xt(e, w),
                                    start=(e == 0), stop=(e == ECH - 1),
                                )
                        emit_pending_e()

                        comb = comb_pool.tile([128, MCH], bf16, tag="comb",
                                              name=f"comb{ki}_{a}")
                        for b, s0, s1 in _segments(off, w):
                            nc.scalar.activation(
                                comb[:, s0:s1], P[:, s0:s1], AF.Relu,
                                bias=att2T_sb[a][:, b:b + 1], scale=1.0,
                            )

                        def make_emm(e_ps=e_ps, a=a, comb=comb, w=w,
                                     e_row=e_row, off=off, gbase=gbase):
                            def emm():
                                nc.tensor.matmul(
                                    e_ps[0:1, :w],
                                    lhsT=late["wfull"][:, a:a + 1],
                                    rhs=comb[:, :w],
                                    start=(a == 0), stop=(a == ACH - 1),
                                )
                                if a == ACH - 1:
                                    nc.vector.tensor_copy(
                                        e_row[0:1, off - gbase:off - gbase + w],
                                        e_ps[0:1, :w])
                            return emm
                        pending_e.append(make_emm())

                    # end of chunk: spread encN prefetch triggers
                    prefetch_xn(4 if ki > 0 else 0)

                    last_of_group = (ki + 1 == len(p1ch)
                                     or p1ch[ki + 1][0] != g)
                    if not last_of_group:
                        continue

                    # transposes of previous groups' A chunks (deps are
                    # long satisfied, so these never stall the PE stream)
                    emit_transposes(at_pool, gbase)

                    # ---- softmax for group g (runs on DVE/ACT in the
                    # shadow of the next group's matmuls) ----
                    emit_pending_e()
                    last_group = (g == GROUPS - 1)
                    dma = nc.sync.dma_start if last_group else nc.gpsimd.dma_start
                    negmax = sm_pool.tile([1, BPG], f32, tag="negmax",
                                          name=f"negmax{g}")
                    sums = sm_pool.tile([1, BPG], f32, tag="sums",
                                        name=f"sums{g}")
                    for i in range(BPG):
                        sl = e_row[0:1, N * i:N * i + N]
                        nc.vector.reduce_max(
                            negmax[0:1, i:i + 1], sl,
                            axis=mybir.AxisListType.X, negate=True)
                        nc.scalar.activation(
                            sl, sl, AF.Exp,
                            bias=negmax[0:1, i:i + 1], scale=1.0,
                            accum_out=sums[0:1, i:i + 1])
                    rec = sm_pool.tile([1, BPG], f32, tag="rec",
                                       name=f"rec{g}")
                    nc.vector.reciprocal(rec[:], sums[:])
                    alpha_b = sm_pool.tile([1, GM], bf16, tag="alpha_b",
                                           name=f"alpha_b{g}")
                    for i in range(BPG):
                        nc.vector.tensor_scalar_mul(
                            e_row[0:1, N * i:N * i + N],
                            e_row[0:1, N * i:N * i + N], rec[0:1, i:i + 1])
                        dma(alpha_out[BPG * g + i:BPG * g + i + 1, :],
                            e_row[0:1, N * i:N * i + N])
                    nc.vector.tensor_copy(alpha_b[:], e_row[:])
                    for i in range(BPG):
                        b = BPG * g + i
                        dma(AT[b:b + 1, N * b:N * b + N],
                            alpha_b[0:1, N * i:N * i + N])

                assert xn_next[0] == nch2

            # ---------------- phase 2 ----------------
            # Chunks whose A tiles are already transposed run first — they
            # keep the PE busy while the last group's softmax completes;
            # the remaining transposes then no longer stall the stream.
            with tc.tile_pool(name="p2psum", bufs=1, space="PSUM") as p2_pool:
                awe_ps = [
                    p2_pool.tile([BPC, 512], f32, tag=f"awe{s}",
                                 name=f"awe_ps{s}")
                    for s in range(ACH)
                ]

                def p2_mms(jlo, jhi):
                    for jj in range(jlo, jhi):
                        off2, w2 = p2ch[jj]
                        xn, At = xn_tiles[jj], A_tiles[jj]
                        for s in range(4):
                            nc.tensor.matmul(
                                awe_ps[s][:],
                                lhsT=At[:w2, :],
                                rhs=xn[:w2, 512 * s:512 * s + 512],
                                start=(jj == 0), stop=(jj == nch2 - 1),
                            )

                ready = a_done[0]
                p2_mms(0, ready)
                # p1 pools are closed here, so the last transposes get a
                # deeper PSUM pool and don't serialize on one slot
                with tc.tile_pool(name="atpsum2", bufs=3,
                                  space="PSUM") as at2_pool:
                    emit_transposes(at2_pool, GM * GROUPS)
                assert a_done[0] == nch2
                p2_mms(ready, nch2)
                awe_sb = consts.tile([BPC, ENC], f32, tag="awe_sb")
                for s in range(4):
                    eng = nc.vector.tensor_copy if s % 2 == 0 else nc.scalar.copy
                    eng(awe_sb[:, 512 * s:512 * s + 512], awe_ps[s][:])
                nc.sync.dma_start(awe_out[:], awe_sb[:])
            at_ctx.__exit__(None, None, None)

    nc.compile()
    return nc


def host_inputs(encoder_out, decoder_hidden, W_enc, b_enc, W_dec, b_dec,
                W_full, b_full):
    """Shard + cast + transpose on the host; returns in_maps for 8 cores."""
    import ml_dtypes
    bf = ml_dtypes.bfloat16

    enc = np.asarray(encoder_out, dtype=np.float32)
    dec = np.asarray(decoder_hidden, dtype=np.float32)
    W_enc = np.asarray(W_enc, dtype=np.float32)
    b_enc = np.asarray(b_enc, dtype=np.float32)
    W_dec = np.asarray(W_dec, dtype=np.float32)
    b_dec = np.asarray(b_dec, dtype=np.float32)
    W_full = np.asarray(W_full, dtype=np.float32)

    wenc_b = W_enc.astype(bf)
    wdec_b = W_dec.astype(bf)
    wfull_b = np.ascontiguousarray(W_full.reshape(ACH, 128).T).astype(bf)
    bsum_b = (b_enc + b_dec).reshape(1, ATT).astype(bf)
    ident_b = np.eye(BPC, dtype=bf)

    in_maps = []
    for c in range(CORES):
        sl = enc[BPC * c:BPC * c + BPC].reshape(M, ENC)
        in_maps.append({
            "encT": np.ascontiguousarray(sl.T).astype(bf),
            "encN": sl.astype(bf),
            "decT": np.ascontiguousarray(dec[BPC * c:BPC * c + BPC].T).astype(bf),
            "wenc": wenc_b,
            "wdec": wdec_b,
            "wfull": wfull_b,
            "bsum": bsum_b,
            "ident": ident_b,
        })
    return in_maps


_NC = None


def kernel(encoder_out, decoder_hidden, W_enc, b_enc, W_dec, b_dec,
           W_full, b_full):
    global _NC
    from concourse.bass_utils import run_bass_kernel_spmd

    if _NC is None:
        _NC = build_graph()
    in_maps = host_inputs(encoder_out, decoder_hidden, W_enc, b_enc,
                          W_dec, b_dec, W_full, b_full)
    res = run_bass_kernel_spmd(_NC, in_maps, core_ids=list(range(CORES)))
    awe = np.concatenate(
        [np.asarray(res.results[c]["awe_out"]) for c in range(CORES)], axis=0
    ).astype(np.float32)
    alpha = np.concatenate(
        [np.asarray(res.results[c]["alpha_out"]) for c in range(CORES)], axis=0
    ).astype(np.float32)
    return awe, alpha


# revision 33
# speedup vs baseline: 1.0698x; 1.0698x over previous
"""Bahdanau attention kernel for 8 TRN2 NeuronCores.

Data-parallel over batch: each core handles 16 of the 128 batches.
Per core (m = 16*196 = 3136 flattened (batch, n) rows), pipelined over
4 groups of 4 batches:
  phase 1(g): att1^T[a, m] = W_enc^T @ enc^T  (bf16 matmuls),
              fused bias+relu on ScalarE (att2 per-batch segments),
              e[m] = W_full^T @ combined  (PE matvec, deferred one
              P-block so it never stalls the in-order PE stream)
  softmax(g): per batch over n on partition 0 (overlaps the next group)
  phase 2:    awe[b, :] = A^T @ enc_flat with A = block-diag(alpha),
              single accumulation pass at the end; encN prefetch triggers
              spread through phase 1.
DMA triggers cost ~0.7us each on the sync engine, so loads are merged:
4 e-chunks per encT trigger, one trigger for wdec/decT.
All heavy matmuls run in bf16 (host-cast); accumulation is f32 PSUM.
"""

import numpy as np

CORES = 8
B, N = 128, 196
ENC, DEC, ATT = 2048, 512, 512
BPC = B // CORES          # 16 batches per core
M = BPC * N               # 3136 flattened rows per core
GROUPS = 4
BPG = BPC // GROUPS       # 4 batches per group
GM = BPG * N              # 784 rows per group
MCH = 512                 # phase-1 moving chunk (PSUM bank limit, f32 out)
ECH = ENC // 128          # 16 contraction chunks
ACH = ATT // 128          # 4 a-chunks
DCH = DEC // 128          # 4 d-chunks
EQ = 4                    # e-chunks per merged encT DMA


def _chunks(base, total, step):
    return [(base + o, min(step, total - o)) for o in range(0, total, step)]


def _segments(off, w):
    """(batch, start, end) segments (chunk-local cols) of flat range."""
    segs = []
    b0, b1 = off // N, (off + w - 1) // N
    for b in range(b0, b1 + 1):
        s0, s1 = max(off, b * N), min(off + w, (b + 1) * N)
        segs.append((b, s0 - off, s1 - off))
    return segs


def build_graph():
    import concourse.bacc as bacc
    import concourse.mybir as mybir
    import concourse.tile as tile

    f32 = mybir.dt.float32
    bf16 = mybir.dt.bfloat16
    AF = mybir.ActivationFunctionType

    nc = bacc.Bacc("TRN2", target_bir_lowering=False, debug=False)

    encT = nc.dram_tensor("encT", [ENC, M], bf16, kind="ExternalInput").ap()
    encN = nc.dram_tensor("encN", [M, ENC], bf16, kind="ExternalInput").ap()
    decT = nc.dram_tensor("decT", [DEC, BPC], bf16, kind="ExternalInput").ap()
    wenc = nc.dram_tensor("wenc", [ENC, ATT], bf16, kind="ExternalInput").ap()
    wdec = nc.dram_tensor("wdec", [DEC, ATT], bf16, kind="ExternalInput").ap()
    wfull = nc.dram_tensor("wfull", [128, ACH], bf16, kind="ExternalInput").ap()
    bsum = nc.dram_tensor("bsum", [1, ATT], bf16, kind="ExternalInput").ap()
    ident = nc.dram_tensor("ident", [BPC, BPC], bf16, kind="ExternalInput").ap()
    alpha_out = nc.dram_tensor("alpha_out", [BPC, N], f32, kind="ExternalOutput").ap()
    awe_out = nc.dram_tensor("awe_out", [BPC, ENC], f32, kind="ExternalOutput").ap()

    p2ch = _chunks(0, M, 128)          # 25 phase-2 chunks
    nch2 = len(p2ch)
    p1ch = []                          # all (group, off, w) phase-1 chunks
    for g in range(GROUPS):
        for off, w in _chunks(GM * g, GM, MCH):
            p1ch.append((g, off, w))

    with tile.TileContext(nc) as tc:
        with (
            tc.tile_pool(name="consts", bufs=1) as consts,
            tc.tile_pool(name="xT", bufs=12) as xt_pool,
            tc.tile_pool(name="xN", bufs=nch2) as xn_pool,
            tc.tile_pool(name="Apool", bufs=nch2) as a_pool,
            tc.tile_pool(name="comb", bufs=6) as comb_pool,
            tc.tile_pool(name="sm", bufs=2) as sm_pool,
        ):
            def load_xt(off, w, tag_sfx):
                """4 merged DMAs, each covering 4 e-chunks of encT cols.
                Returns a slicer f(e, w) -> rhs AP."""
                quarters = []
                for q in range(EQ):
                    t = xt_pool.tile([128, EQ * MCH], bf16, tag="xt",
                                     name=f"xt_{tag_sfx}_{q}")
                    src = encT[512 * q:512 * q + 512, off:off + w].rearrange(
                        "(e p) c -> p e c", p=128)
                    dst = t[:].rearrange("p (e c) -> p e c", e=EQ)[:, :, 0:w]
                    nc.sync.dma_start(dst, src)
                    quarters.append(t)
                return lambda e, w: quarters[e // EQ][
                    :, MCH * (e % EQ):MCH * (e % EQ) + w]

            # PE warmup: the HAM clock gate needs ~3.4us of sustained
            # activity to lift the 1.2GHz throttle. Dummy matmuls on a
            # memset tile run while the first real loads are in flight, so
            # the real stream starts at 2.4GHz. The memset is the first DVE
            # instruction so the warmup isn't delayed by other memsets.
            wu_sb = consts.tile([128, MCH], bf16, tag="wu")
            nc.vector.memset(wu_sb[:], 0.25)
            with tc.tile_pool(name="wupsum", bufs=1, space="PSUM") as wu_pool:
                wu_ps = wu_pool.tile([128, MCH], f32, tag="wups")
                for _ in range(7):
                    nc.tensor.matmul(wu_ps[:], lhsT=wu_sb[:, 0:128],
                                     rhs=wu_sb[:], start=True, stop=True)

            # ---- startup: first e-chunks land as small single DMAs so the
            # first matmul starts as early as possible ----
            wenc_fine = []
            xt0_fine = []
            for e in range(EQ):
                t = consts.tile([128, ATT], bf16, tag=f"wencf{e}",
                                name=f"wenc_fine{e}")
                nc.sync.dma_start(t[:], wenc[128 * e:128 * e + 128, :])
                wenc_fine.append(t)
                t2 = xt_pool.tile([128, MCH], bf16, tag="xtf", bufs=4,
                                  name=f"xt0_fine{e}")
                nc.sync.dma_start(t2[:], encT[128 * e:128 * e + 128, 0:MCH])
                xt0_fine.append(t2)

            wenc_sb = [None]
            xt0_q = [None]
            for q in range(1, EQ):
                t = consts.tile([128, EQ * ATT], bf16, tag=f"wenc{q}",
                                name=f"wenc_sb{q}")
                src = wenc[512 * q:512 * q + 512, :].rearrange(
                    "(e p) c -> p e c", p=128)
                dst = t[:].rearrange("p (e c) -> p e c", e=EQ)
                nc.sync.dma_start(dst, src)
                wenc_sb.append(t)
                tq = xt_pool.tile([128, EQ * MCH], bf16, tag="xt",
                                  name=f"xt0_q{q}")
                srcq = encT[512 * q:512 * q + 512, 0:MCH].rearrange(
                    "(e p) c -> p e c", p=128)
                dstq = tq[:].rearrange("p (e c) -> p e c", e=EQ)
                nc.sync.dma_start(dstq, srcq)
                xt0_q.append(tq)

            def wenc_slice(e, a):
                if e < EQ:
                    return wenc_fine[e][:, 128 * a:128 * a + 128]
                return wenc_sb[e // EQ][:, ATT * (e % EQ) + 128 * a:
                                        ATT * (e % EQ) + 128 * a + 128]

            def xt0_slice(e, w):
                if e < EQ:
                    return xt0_fine[e][:, 0:w]
                return xt0_q[e // EQ][:, MCH * (e % EQ):MCH * (e % EQ) + w]

            decT_sb = consts.tile([128, DCH * BPC], bf16, tag="decT")
            nc.sync.dma_start(
                decT_sb[:].rearrange("p (d c) -> p d c", d=DCH),
                decT[:].rearrange("(d p) c -> p d c", p=128))
            wdec_sb = consts.tile([128, DCH * ATT], bf16, tag="wdec")
            nc.sync.dma_start(
                wdec_sb[:].rearrange("p (d c) -> p d c", d=DCH),
                wdec[:].rearrange("(d p) c -> p d c", p=128))

            late = {}
            late["bsum"] = consts.tile([1, ATT], bf16, tag="bsum",
                                       name="bsum_sb")
            nc.sync.dma_start(late["bsum"][:], bsum[:])
            late["wfull"] = consts.tile([128, ACH], bf16, tag="wfull",
                                        name="wfull_sb")
            nc.sync.dma_start(late["wfull"][:], wfull[:])

            ones_sb = consts.tile([1, BPC], bf16, tag="ones")
            nc.vector.memset(ones_sb[:], 1.0)
            AT = consts.tile([BPC, M], bf16, tag="AT")
            nc.vector.memset(AT[:], 0.0)

            # ident is needed only by the group-1 transposes — issue its
            # trigger after chunk 1's encT loads
            def emit_late_consts():
                late["ident"] = consts.tile([BPC, BPC], bf16, tag="ident",
                                            name="ident_sb")
                nc.sync.dma_start(late["ident"][:], ident[:])

            A_tiles = [None] * nch2
            xn_tiles = [None] * nch2
            att2T_sb = []
            pending_e = []          # deferred e-matmuls
            xn_next = [0]           # next phase-2 chunk to prefetch
            a_done = [0]            # transposes emitted so far

            def emit_pending_e():
                while pending_e:
                    pending_e.pop(0)()

            def emit_att2():
                with tc.tile_pool(name="t2psum", bufs=1,
                                  space="PSUM") as t2_pool:
                    for aa in range(ACH):
                        t2ps = t2_pool.tile([128, BPC], f32, tag="t2",
                                            name=f"t2ps{aa}")
                        for d in range(DCH):
                            nc.tensor.matmul(
                                t2ps[:],
                                lhsT=wdec_sb[:, ATT * d + 128 * aa:
                                             ATT * d + 128 * aa + 128],
                                rhs=decT_sb[:, BPC * d:BPC * d + BPC],
                                start=(d == 0), stop=False,
                            )
                        nc.tensor.matmul(
                            t2ps[:],
                            lhsT=late["bsum"][0:1, 128 * aa:128 * aa + 128],
                            rhs=ones_sb[0:1, :],
                            start=False, stop=True,
                        )
                        t = consts.tile([128, BPC], f32, tag=f"att2T{aa}",
                                        name=f"att2T_sb{aa}")
                        nc.vector.tensor_copy(t[:], t2ps[:])
                        att2T_sb.append(t)

            def prefetch_xn(count):
                while count > 0 and xn_next[0] < nch2:
                    jj = xn_next[0]
                    off2, w2 = p2ch[jj]
                    t = xn_pool.tile([128, ENC], bf16, tag="xn",
                                     name=f"xn{jj}")
                    nc.sync.dma_start(t[:w2, :], encN[off2:off2 + w2, :])
                    xn_tiles[jj] = t
                    xn_next[0] += 1
                    count -= 1

            def emit_transposes(at_pool, lim):
                while (a_done[0] < nch2
                       and p2ch[a_done[0]][0] + p2ch[a_done[0]][1] <= lim):
                    off2, w2 = p2ch[a_done[0]]
                    aps = at_pool.tile([128, BPC], bf16, tag="Aps",
                                       name=f"aps{a_done[0]}")
                    nc.tensor.transpose(
                        aps[:w2, :], AT[:, off2:off2 + w2], late["ident"][:])
                    t = a_pool.tile([128, BPC], bf16, tag="A",
                                    name=f"A_sb{a_done[0]}")
                    nc.vector.tensor_copy(t[:w2, :], aps[:w2, :])
                    A_tiles[a_done[0]] = t
                    a_done[0] += 1

            at_ctx = tc.tile_pool(name="atpsum", bufs=1, space="PSUM")
            at_pool = at_ctx.__enter__()
            with (
                tc.tile_pool(name="p1psum", bufs=4, space="PSUM") as p1_pool,
                tc.tile_pool(name="epsum", bufs=2, space="PSUM") as e_pool,
            ):
                e_rows = {}
                for ki, (g, off, w) in enumerate(p1ch):
                    gbase = GM * g
                    if ki == 0:
                        xt = xt0_slice
                    else:
                        xt = load_xt(off, w, f"k{ki}")
                        if ki == 1:
                            emit_late_consts()
                    if g not in e_rows:
                        e_rows[g] = sm_pool.tile([1, GM], f32, tag="e_row",
                                                 name=f"e_row{g}")
                    e_row = e_rows[g]
                    e_ps = e_pool.tile([1, MCH], f32, tag="eps",
                                       name=f"eps{ki}")
                    if ki == 0:
                        # e-outer: each e-step needs only 256KB of fresh
                        # data, so the PE never outruns the startup DMAs
                        Ps = [p1_pool.tile([128, MCH], f32, tag="P",
                                           name=f"P0_{a}")
                              for a in range(ACH)]
                        for e in range(ECH):
                            for a in range(ACH):
                                nc.tensor.matmul(
                                    Ps[a][:, :w],
                                    lhsT=wenc_slice(e, a),
                                    rhs=xt(e, w),
                                    start=(e == 0), stop=(e == ECH - 1),
                                )
                            if e == 7:
                                emit_att2()
                    for a in range(ACH):
                        if ki == 0:
                            P = Ps[a]
                        else:
                            P = p1_pool.tile([128, MCH], f32, tag="P",
                                             name=f"P{ki}_{a}")
                            for e in range(ECH):
                                nc.tensor.matmul(
                                    P[:, :w],
                                    lhsT=wenc_slice(e, a),
                                    rhs=xt(e, w),
                                    start=(e == 0), stop=(e == ECH - 1),
                                )
                        emit_pending_e()

                        comb = comb_pool.tile([128, MCH], bf16, tag="comb",
                                              name=f"comb{ki}_{a}")
                        for b, s0, s1 in _segments(off, w):
                            nc.scalar.activation(
                                comb[:, s0:s1], P[:, s0:s1], AF.Relu,
                                bias=att2T_sb[a][:, b:b + 1], scale=1.0,
                            )

                        def make_emm(e_ps=e_ps, a=a, comb=comb, w=w,
                                     e_row=e_row, off=off, gbase=gbase):
                            def emm():
                                nc.tensor.matmul(
                                    e_ps[0:1, :w],
                                    lhsT=late["wfull"][:, a:a + 1],
                                    rhs=comb[:, :w],
                                    start=(a == 0), stop=(a == ACH - 1),
                                )
                                if a == ACH - 1:
                                    nc.vector.tensor_copy(
                                        e_row[0:1, off - gbase:off - gbase + w],
                                        e_ps[0:1, :w])
                            return emm
                        pending_e.append(make_emm())

                    # end of chunk: spread encN prefetch triggers
                    prefetch_xn(4 if ki > 0 else 0)

                    last_of_group = (ki + 1 == len(p1ch)
                                     or p1ch[ki + 1][0] != g)
                    if not last_of_group:
                        continue

                    # transposes of previous groups' A chunks (deps are
                    # long satisfied, so these never stall the PE stream)
                    emit_transposes(at_pool, gbase)

                    # ---- softmax for group g (runs on DVE/ACT in the
                    # shadow of the next group's matmuls) ----
                    emit_pending_e()
                    last_group = (g == GROUPS - 1)
                    dma = nc.sync.dma_start if last_group else nc.gpsimd.dma_start
                    negmax = sm_pool.tile([1, BPG], f32, tag="negmax",
                                          name=f"negmax{g}")
                    sums = sm_pool.tile([1, BPG], f32, tag="sums",
                                        name=f"sums{g}")
                    for i in range(BPG):
                        sl = e_row[0:1, N * i:N * i + N]
                        nc.vector.reduce_max(
                            negmax[0:1, i:i + 1], sl,
                            axis=mybir.AxisListType.X, negate=True)
                        nc.scalar.activation(
                            sl, sl, AF.Exp,
                            bias=negmax[0:1, i:i + 1], scale=1.0,
                            accum_out=sums[0:1, i:i + 1])
                    rec = sm_pool.tile([1, BPG], f32, tag="rec",
                                       name=f"rec{g}")
                    nc.vector.reciprocal(rec[:], sums[:])
                    alpha_b = sm_pool.tile([1, GM], bf16, tag="alpha_b",
                                           name=f"alpha_b{g}")
                    for i in range(BPG):
                        nc.vector.tensor_scalar_mul(
                            e_row[0:1, N * i:N * i + N],
                            e_row[0:1, N * i:N * i + N], rec[0:1, i:i + 1])
                        dma(alpha_out[BPG * g + i:BPG * g + i + 1, :],
                            e_row[0:1, N * i:N * i + N])
                    nc.vector.tensor_copy(alpha_b[:], e_row[:])
                    for i in range(BPG):
                        b = BPG * g + i
                        dma(AT[b:b + 1, N * b:N * b + N],
                            alpha_b[0:1, N * i:N * i + N])

                assert xn_next[0] == nch2

            # ---------------- phase 2 ----------------
            # Chunks whose A tiles are already transposed run first — they
            # keep the PE busy while the last group's softmax completes;
            # the remaining transposes then no longer stall the stream.
            with tc.tile_pool(name="p2psum", bufs=1, space="PSUM") as p2_pool:
                awe_ps = [
                    p2_pool.tile([BPC, 512], f32, tag=f"awe{s}",
                                 name=f"awe_ps{s}")
                    for s in range(ACH)
                ]

                def p2_mms(jlo, jhi):
                    for jj in range(jlo, jhi):
                        off2, w2 = p2ch[jj]
                        xn, At = xn_tiles[jj], A_tiles[jj]
                        for s in range(4):
                            nc.tensor.matmul(
                                awe_ps[s][:],
                                lhsT=At[:w2, :],
                                rhs=xn[:w2, 512 * s:512 * s + 512],
                                start=(jj == 0), stop=(jj == nch2 - 1),
                            )

                ready = a_done[0]
                p2_mms(0, ready)
                # p1 pools are closed here, so the last transposes get a
                # deeper PSUM pool and don't serialize on one slot
                with tc.tile_pool(name="atpsum2", bufs=3,
                                  space="PSUM") as at2_pool:
                    emit_transposes(at2_pool, GM * GROUPS)
                assert a_done[0] == nch2
                p2_mms(ready, nch2)
                awe_sb = consts.tile([BPC, ENC], f32, tag="awe_sb")
                for s in range(4):
                    eng = nc.vector.tensor_copy if s % 2 == 0 else nc.scalar.copy
                    eng(awe_sb[:, 512 * s:512 * s + 512], awe_ps[s][:])
                nc.sync.dma_start(awe_out[:], awe_sb[:])
            at_ctx.__exit__(None, None, None)

    nc.compile()
    return nc


def host_inputs(encoder_out, decoder_hidden, W_enc, b_enc, W_dec, b_dec,
                W_full, b_full):
    """Shard + cast + transpose on the host; returns in_maps for 8 cores."""
    import ml_dtypes
    bf = ml_dtypes.bfloat16

    enc = np.asarray(encoder_out, dtype=np.float32)
    dec = np.asarray(decoder_hidden, dtype=np.float32)
    W_enc = np.asarray(W_enc, dtype=np.float32)
    b_enc = np.asarray(b_enc, dtype=np.float32)
    W_dec = np.asarray(W_dec, dtype=np.float32)
    b_dec = np.asarray(b_dec, dtype=np.float32)
    W_full = np.asarray(W_full, dtype=np.float32)

    wenc_b = W_enc.astype(bf)
    wdec_b = W_dec.astype(bf)
    wfull_b = np.ascontiguousarray(W_full.reshape(ACH, 128).T).astype(bf)
    bsum_b = (b_enc + b_dec).reshape(1, ATT).astype(bf)
    ident_b = np.eye(BPC, dtype=bf)

    in_maps = []
    for c in range(CORES):
        sl = enc[BPC * c:BPC * c + BPC].reshape(M, ENC)
        in_maps.append({
            "encT": np.ascontiguousarray(sl.T).astype(bf),
            "encN": sl.astype(bf),
            "decT": np.ascontiguousarray(dec[BPC * c:BPC * c + BPC].T).astype(bf),
            "wenc": wenc_b,
            "wdec": wdec_b,
            "wfull": wfull_b,
            "bsum": bsum_b,
            "ident": ident_b,
        })
    return in_maps


_NC = None


def kernel(encoder_out, decoder_hidden, W_enc, b_enc, W_dec, b_dec,
           W_full, b_full):
    global _NC
    from concourse.bass_utils import run_bass_kernel_spmd

    if _NC is None:
        _NC = build_graph()
    in_maps = host_inputs(encoder_out, decoder_hidden, W_enc, b_enc,
                          W_dec, b_dec, W_full, b_full)
    res = run_bass_kernel_spmd(_NC, in_maps, core_ids=list(range(CORES)))
    awe = np.concatenate(
        [np.asarray(res.results[c]["awe_out"]) for c in range(CORES)], axis=0
    ).astype(np.float32)
    alpha = np.concatenate(
        [np.asarray(res.results[c]["alpha_out"]) for c in range(CORES)], axis=0
    ).astype(np.float32)
    return awe, alpha
